# revision 19
# baseline (speedup 1.0000x reference)
"""Trainium2 Bass kernel for an additive-coupling layer (NICE-style).

Reference math (mask_config=1, forward):
    x:[4096,2048] -> x1 = x[:, 0::2], x2 = x[:, 1::2]           (each [4096,1024])
    h = relu(x2 @ W_in + b_in)                                   [4096,4096]
    h = relu(h @ W_h[i] + b_h[i])  for i in 0..3
    out = h @ W_out + b_out                                      [4096,1024]
    y[:, 0::2] = x1 + out ; y[:, 1::2] = x2
    returns (y, log_det_J)  (log_det_J passes through unchanged)

Distribution: pure data-parallel over the batch. Each of the 8 NeuronCores
gets 512 rows and a replicated copy of the weights; no collectives.

Per-core kernel (MODE="fp8", the default): activations stay feature-major
(features on SBUF partitions, batch on the free dim) so every layer is a
chain of fp8e4m3 DoubleRow matmuls — 128x256 stationary (weight pair-tile)
x 256x512 moving (activation) — accumulating fp32 in PSUM at 2 MACs per PE
cell per cycle (157 TF/s peak). Weights are quantized with power-of-two
scales and pre-arranged on the host into per-output-tile strips so each
strip is one contiguous DMA; the ScalarE drain fuses bias + rescale + relu
+ fp8 quantization of the next layer's input. The output layer instead
makes the activations stationary so its PSUM result lands batch-major;
one DVE op per tile fuses the descale with the x1 add (b_out is folded
into the host-marshaled x1), and the y rows DMA out per batch-tile. A
burst of identity transposes at t=0 warms the PE clock gate (HAM) while
the first DMAs land.

Measured on one TRN2 chip: ~526 us NEFF exec (vs ~494 us pure-matmul
floor at fp8 peak); the bf16 variant (MODE="bf16") measures ~1059 us.
fp8 end-to-end error on y is ~4e-4 relative: the coupling output rides
on the exact x1 passthrough at ~1% of its scale, diluting GEMM error.
"""

import numpy as np
import ml_dtypes

BF16 = ml_dtypes.bfloat16

B = 4096
D = 2048
HALF = 1024
MID = 4096
N_HIDDEN = 4
NCORES = 8
BL = B // NCORES          # 512 rows per core
KT_IN = HALF // 128       # 8   k-tiles, input layer
KT_MID = MID // 128       # 32  k-tiles, hidden layers
MT_MID = MID // 128       # 32  m-tiles (output feature tiles), hidden layers
MT_OUT = HALF // 128      # 8   m-tiles, output layer
NBT = BL // 128           # 4   batch tiles per core

_CACHE = {}

# fp8 (e4m3, DoubleRow) quantization scales — powers of two. Weight tensors
# scale to absmax 128; activations to absmax <= ~100 (TRN fp8 overflows to
# Inf at 240, so keep >2x headroom). Derived from the fp32 forward pass.
SW_IN = 4096.0
SW_H = 8192.0
SW_OUT = 8192.0
SA = [16.0, 32.0, 64.0, 128.0, 256.0, 512.0]
KT2_IN = HALF // 256      # 4   k-pair-tiles, input layer
KT2_MID = MID // 256      # 16  k-pair-tiles, hidden layers
FP8 = ml_dtypes.float8_e4m3

MODE = "fp8"              # "fp8" or "bf16"


def _build_nc_fp8():
    """fp8e4m3 DoubleRow variant: 2 fp8 MACs per PE cell per cycle.

    Logical contraction index k_eff = kt*256 + j*128 + p maps to
    (k-pair-tile kt, pair j, partition p); lhsT/rhs APs are [128, 2, F]
    with the pair on dim 1 (= two adjacent 128-feature tiles), so the
    producing layer's per-m-tile ACT drain writes stay contiguous.
    """
    from contextlib import ExitStack

    import concourse.bacc as bacc
    import concourse.mybir as mybir
    import concourse.tile as tile

    f32 = mybir.dt.float32
    fp8 = mybir.dt.float8e4
    AF = mybir.ActivationFunctionType
    DR = mybir.MatmulPerfMode.DoubleRow

    nc = bacc.Bacc()

    x2t_d = nc.declare_dram_parameter("x2t", [128, KT2_IN, 2, BL], fp8, isOutput=False)
    xloc_d = nc.declare_dram_parameter("xloc", [NBT, 128, HALF, 2], f32, isOutput=False)
    win_d = nc.declare_dram_parameter("win", [MT_MID, 128, KT2_IN, 2, 128], fp8, isOutput=False)
    wh_d = nc.declare_dram_parameter("wh", [N_HIDDEN * MT_MID, 128, KT2_MID, 2, 128], fp8, isOutput=False)
    woutt_d = nc.declare_dram_parameter("woutt", [128, KT2_MID, 2, HALF], fp8, isOutput=False)
    bin_d = nc.declare_dram_parameter("bin", [128, MT_MID], f32, isOutput=False)
    bh_d = nc.declare_dram_parameter("bh", [128, N_HIDDEN * MT_MID], f32, isOutput=False)
    ident_d = nc.declare_dram_parameter("ident", [128, 128], f32, isOutput=False)
    out_d = nc.declare_dram_parameter("out", [NBT, 128, HALF, 2], f32, isOutput=True)

    with tile.TileContext(nc) as tc, ExitStack() as ctx:
        const = ctx.enter_context(tc.tile_pool(name="const", bufs=1))
        xp = ctx.enter_context(tc.tile_pool(name="xp", bufs=1))
        hp = ctx.enter_context(tc.tile_pool(name="hp", bufs=2))
        wp = ctx.enter_context(tc.tile_pool(name="wp", bufs=6))
        op = ctx.enter_context(tc.tile_pool(name="op", bufs=1))
        pp = ctx.enter_context(tc.tile_pool(name="pp", bufs=6, space="PSUM"))
        tp = ctx.enter_context(tc.tile_pool(name="tp", bufs=1, space="PSUM"))

        # identity comes via DMA: gpsimd (make_identity) pays a ~7us library
        # load at kernel start, which would delay the PE warmup.
        ident = const.tile([128, 128], f32)
        nc.sync.dma_start(ident[:], ident_d[:])
        x2t = const.tile([128, KT2_IN, 2, BL], fp8)
        nc.sync.dma_start(x2t[:], x2t_d[:])
        bins = const.tile([128, MT_MID], f32)
        nc.sync.dma_start(bins[:], bin_d[:])
        bhs = const.tile([128, N_HIDDEN * MT_MID], f32)
        nc.sync.dma_start(bhs[:], bh_d[:])
        # Warm the PE (HAM un-throttles after ~3.4us of sustained activity)
        # with dependency-free identity transposes while input DMAs land.
        warm = tp.tile([128, 128], f32, name="warm", tag="tp")
        for _ in range(20):
            nc.tensor.transpose(warm[:], ident[:], ident[:])

        # input layer
        sc0 = SA[1] / (SW_IN * SA[0])
        h = hp.tile([128, KT2_MID, 2, BL], fp8, name="h0", tag="h")
        for mt in range(MT_MID):
            w = wp.tile([128, KT2_IN, 2, 128], fp8, name=f"w0_{mt}", tag="w")
            nc.sync.dma_start(w[:], win_d[mt])
            ps = pp.tile([128, BL], f32, name=f"ps0_{mt}", tag="ps")
            for kt in range(KT2_IN):
                nc.tensor.matmul(
                    ps[:], w[:, kt], x2t[:, kt],
                    start=(kt == 0), stop=(kt == KT2_IN - 1), perf_mode=DR,
                )
            nc.scalar.activation(
                h[:, mt // 2, mt % 2, :], ps[:], AF.Relu,
                bias=bins[:, mt:mt + 1], scale=sc0,
            )

        # x rows are only needed for the final add — spread the four loads
        # across the hidden layers so they never delay a weight-strip DMA.
        xsb = xp.tile([128, NBT, HALF, 2], f32)

        woutt = op.tile([128, KT2_MID, 2, HALF], fp8)

        # hidden layers
        for layer in range(N_HIDDEN):
            nc.sync.dma_start(xsb[:, layer], xloc_d[layer])
            if layer == 1:
                nc.sync.dma_start(woutt[:], woutt_d[:])
            scl = SA[layer + 2] / (SW_H * SA[layer + 1])
            h2 = hp.tile([128, KT2_MID, 2, BL], fp8, name=f"h{layer + 1}", tag="h")
            for mt in range(MT_MID):
                lm = layer * MT_MID + mt
                w = wp.tile([128, KT2_MID, 2, 128], fp8, name=f"wh{lm}", tag="w")
                nc.sync.dma_start(w[:], wh_d[lm])
                ps = pp.tile([128, BL], f32, name=f"ps{layer + 1}_{mt}", tag="ps")
                for kt in range(KT2_MID):
                    nc.tensor.matmul(
                        ps[:], w[:, kt], h[:, kt],
                        start=(kt == 0), stop=(kt == KT2_MID - 1), perf_mode=DR,
                    )
                nc.scalar.activation(
                    h2[:, mt // 2, mt % 2, :], ps[:], AF.Relu,
                    bias=bhs[:, lm:lm + 1], scale=scl,
                )
            h = h2

        # output layer, batch-major: activations become the stationary
        # operand so psum[b, f] needs no transpose. b_out is pre-added into
        # the host-marshaled x1 columns; the PSUM descale and the x1 add are
        # fused into one DVE scalar_tensor_tensor per (bt, chunk).
        sco = 1.0 / (SW_OUT * SA[5])
        for bt in range(NBT):
            for c in range(2):
                ps = pp.tile([128, BL], f32, name=f"pso_{bt}_{c}", tag="ps")
                for kt in range(KT2_MID):
                    nc.tensor.matmul(
                        ps[:],
                        h[:, kt, :, bt * 128:(bt + 1) * 128],
                        woutt[:, kt, :, c * 512:(c + 1) * 512],
                        start=(kt == 0), stop=(kt == KT2_MID - 1), perf_mode=DR,
                    )
                dst = xsb[:, bt, c * 512:(c + 1) * 512, 0]
                nc.vector.scalar_tensor_tensor(
                    dst, ps[:], sco, dst,
                    op0=mybir.AluOpType.mult, op1=mybir.AluOpType.add,
                )
            nc.sync.dma_start(out_d[bt], xsb[:, bt])

    nc.compile()
    return nc


def _q8(v):
    return np.clip(v, -240.0, 240.0).astype(FP8)


def _marshal_fp8(x, W_in, b_in, W_h, b_h, W_out, b_out):
    """fp8 host prep: quantized pair-block weight strips + input shards."""
    x = np.ascontiguousarray(np.asarray(x, dtype=np.float32))
    W_in = np.asarray(W_in, dtype=np.float32)
    b_in = np.asarray(b_in, dtype=np.float32)
    W_h = np.asarray(W_h, dtype=np.float32)
    b_h = np.asarray(b_h, dtype=np.float32)
    W_out = np.asarray(W_out, dtype=np.float32)
    b_out = np.asarray(b_out, dtype=np.float32)

    # strips: [mt, partition=k, kt, pair j, m]; element = W[kt*256+j*128+k, mt*128+m]
    win = _q8(np.ascontiguousarray(
        (W_in * SW_IN).reshape(KT2_IN, 2, 128, MT_MID, 128).transpose(3, 2, 0, 1, 4)
    ))
    wh = _q8(np.ascontiguousarray(
        (W_h * SW_H).reshape(N_HIDDEN, KT2_MID, 2, 128, MT_MID, 128)
        .transpose(0, 4, 3, 1, 2, 5)
    ).reshape(N_HIDDEN * MT_MID, 128, KT2_MID, 2, 128))
    woutt = _q8(np.ascontiguousarray(
        (W_out * SW_OUT).reshape(KT2_MID, 2, 128, HALF).transpose(2, 0, 1, 3)
    ))
    bin_ = np.ascontiguousarray((b_in * SA[1]).reshape(MT_MID, 128).T)
    bh = np.ascontiguousarray(
        (b_h * np.array(SA[2:6], np.float32)[:, None]).reshape(N_HIDDEN * MT_MID, 128).T
    )
    ident = np.eye(128, dtype=np.float32)

    in_maps = []
    for c in range(NCORES):
        xc = x[c * BL:(c + 1) * BL].copy()
        x2t = _q8(np.ascontiguousarray(
            (xc[:, 1::2] * SA[0]).T.reshape(KT2_IN, 2, 128, BL).transpose(2, 0, 1, 3)
        ))
        xc[:, 0::2] += b_out          # fold output bias into the x1 passthrough
        xloc = np.ascontiguousarray(xc).reshape(NBT, 128, HALF, 2)
        in_maps.append({
            "x2t": x2t, "xloc": xloc,
            "win": win, "wh": wh, "woutt": woutt,
            "bin": bin_, "bh": bh, "ident": ident,
        })
    return in_maps


def _build_nc():
    """Build the (single, SPMD-identical) Bass graph for one core."""
    from contextlib import ExitStack

    import concourse.bacc as bacc
    import concourse.mybir as mybir
    import concourse.tile as tile
    from concourse import masks

    f32 = mybir.dt.float32
    bf16 = mybir.dt.bfloat16
    AF = mybir.ActivationFunctionType

    nc = bacc.Bacc()

    x2t_d = nc.declare_dram_parameter("x2t", [128, KT_IN * BL], bf16, isOutput=False)
    xloc_d = nc.declare_dram_parameter("xloc", [NBT, 128, HALF, 2], f32, isOutput=False)
    win_d = nc.declare_dram_parameter("win", [MT_MID, 128, KT_IN * 128], bf16, isOutput=False)
    wh_d = nc.declare_dram_parameter("wh", [N_HIDDEN * MT_MID, 128, KT_MID * 128], bf16, isOutput=False)
    wout_d = nc.declare_dram_parameter("wout", [MT_OUT, 128, KT_MID * 128], bf16, isOutput=False)
    bin_d = nc.declare_dram_parameter("bin", [128, MT_MID], f32, isOutput=False)
    bh_d = nc.declare_dram_parameter("bh", [128, N_HIDDEN * MT_MID], f32, isOutput=False)
    bout_d = nc.declare_dram_parameter("bout", [128, MT_OUT], f32, isOutput=False)
    out_d = nc.declare_dram_parameter("out", [NBT, 128, HALF, 2], f32, isOutput=True)

    with tile.TileContext(nc) as tc, ExitStack() as ctx:
        const = ctx.enter_context(tc.tile_pool(name="const", bufs=1))
        xp = ctx.enter_context(tc.tile_pool(name="xp", bufs=1))
        hp = ctx.enter_context(tc.tile_pool(name="hp", bufs=2))
        wp = ctx.enter_context(tc.tile_pool(name="wp", bufs=3))
        op = ctx.enter_context(tc.tile_pool(name="op", bufs=1))
        pp = ctx.enter_context(tc.tile_pool(name="pp", bufs=4, space="PSUM"))
        tp = ctx.enter_context(tc.tile_pool(name="tp", bufs=2, space="PSUM"))

        x2t = const.tile([128, KT_IN * BL], bf16)
        nc.sync.dma_start(x2t[:], x2t_d[:])
        xsb = xp.tile([128, NBT, HALF, 2], f32)
        for bt in range(NBT):
            nc.sync.dma_start(xsb[:, bt], xloc_d[bt])
        bins = const.tile([128, MT_MID], f32)
        nc.sync.dma_start(bins[:], bin_d[:])
        bhs = const.tile([128, N_HIDDEN * MT_MID], f32)
        nc.sync.dma_start(bhs[:], bh_d[:])
        bouts = const.tile([128, MT_OUT], f32)
        nc.sync.dma_start(bouts[:], bout_d[:])
        ident = const.tile([128, 128], f32)
        masks.make_identity(nc, ident[:])

        # input layer: h = relu(x2 @ W_in + b_in), feature-major
        h = hp.tile([128, MT_MID * BL], bf16, name="h0", tag="h")
        for mt in range(MT_MID):
            w = wp.tile([128, KT_IN * 128], bf16, name=f"w0_{mt}", tag="w")
            nc.sync.dma_start(w[:], win_d[mt])
            ps = pp.tile([128, BL], f32, name=f"ps0_{mt}", tag="ps")
            for kt in range(KT_IN):
                nc.tensor.matmul(
                    ps[:],
                    w[:, kt * 128:(kt + 1) * 128],
                    x2t[:, kt * BL:(kt + 1) * BL],
                    start=(kt == 0),
                    stop=(kt == KT_IN - 1),
                )
            nc.scalar.activation(
                h[:, mt * BL:(mt + 1) * BL], ps[:], AF.Relu, bias=bins[:, mt:mt + 1]
            )

        # hidden layers
        for layer in range(N_HIDDEN):
            h2 = hp.tile([128, MT_MID * BL], bf16, name=f"h{layer + 1}", tag="h")
            for mt in range(MT_MID):
                lm = layer * MT_MID + mt
                w = wp.tile([128, KT_MID * 128], bf16, name=f"wh{lm}", tag="w")
                nc.sync.dma_start(w[:], wh_d[lm])
                ps = pp.tile([128, BL], f32, name=f"ps{layer + 1}_{mt}", tag="ps")
                for kt in range(KT_MID):
                    nc.tensor.matmul(
                        ps[:],
                        w[:, kt * 128:(kt + 1) * 128],
                        h[:, kt * BL:(kt + 1) * BL],
                        start=(kt == 0),
                        stop=(kt == KT_MID - 1),
                    )
                nc.scalar.activation(
                    h2[:, mt * BL:(mt + 1) * BL], ps[:], AF.Relu, bias=bhs[:, lm:lm + 1]
                )
            h = h2

        # output layer: out = h @ W_out + b_out (no relu), fp32, feature-major
        outT = op.tile([128, MT_OUT * BL], f32)
        for mt in range(MT_OUT):
            w = wp.tile([128, KT_MID * 128], bf16, name=f"wo_{mt}", tag="w")
            nc.sync.dma_start(w[:], wout_d[mt])
            ps = pp.tile([128, BL], f32, name=f"pso_{mt}", tag="ps")
            for kt in range(KT_MID):
                nc.tensor.matmul(
                    ps[:],
                    w[:, kt * 128:(kt + 1) * 128],
                    h[:, kt * BL:(kt + 1) * BL],
                    start=(kt == 0),
                    stop=(kt == KT_MID - 1),
                )
            nc.scalar.activation(
                outT[:, mt * BL:(mt + 1) * BL], ps[:], AF.Identity,
                bias=bouts[:, mt:mt + 1],
            )

        # transpose out back to batch-major and add into even columns of x
        for mt in range(MT_OUT):
            for bt in range(NBT):
                t = tp.tile([128, 128], f32, name=f"t{mt}_{bt}", tag="tp")
                nc.tensor.transpose(
                    t[:], outT[:, mt * BL + bt * 128: mt * BL + (bt + 1) * 128],
                    ident[:],
                )
                dst = xsb[:, bt, mt * 128:(mt + 1) * 128, 0]
                nc.vector.tensor_add(dst, dst, t[:])

        for bt in range(NBT):
            nc.sync.dma_start(out_d[bt], xsb[:, bt])

    nc.compile()
    return nc


def _marshal(x, W_in, b_in, W_h, b_h, W_out, b_out):
    """Host-side layout prep: bf16 weight strips + per-core input shards."""
    x = np.ascontiguousarray(np.asarray(x, dtype=np.float32))
    W_in = np.asarray(W_in, dtype=np.float32)
    b_in = np.asarray(b_in, dtype=np.float32)
    W_h = np.asarray(W_h, dtype=np.float32)
    b_h = np.asarray(b_h, dtype=np.float32)
    W_out = np.asarray(W_out, dtype=np.float32)
    b_out = np.asarray(b_out, dtype=np.float32)

    # weight strips: strip[mt] laid out [partition=k_in, k_tile, m_in]
    win = np.ascontiguousarray(
        W_in.reshape(KT_IN, 128, MT_MID, 128).transpose(2, 1, 0, 3)
    ).reshape(MT_MID, 128, KT_IN * 128).astype(BF16)
    wh = np.ascontiguousarray(
        W_h.reshape(N_HIDDEN, KT_MID, 128, MT_MID, 128).transpose(0, 3, 2, 1, 4)
    ).reshape(N_HIDDEN * MT_MID, 128, KT_MID * 128).astype(BF16)
    wout = np.ascontiguousarray(
        W_out.reshape(KT_MID, 128, MT_OUT, 128).transpose(2, 1, 0, 3)
    ).reshape(MT_OUT, 128, KT_MID * 128).astype(BF16)
    bin_ = np.ascontiguousarray(b_in.reshape(MT_MID, 128).T)
    bh = np.ascontiguousarray(b_h.reshape(N_HIDDEN * MT_MID, 128).T)
    bout = np.ascontiguousarray(b_out.reshape(MT_OUT, 128).T)

    in_maps = []
    for c in range(NCORES):
        xc = x[c * BL:(c + 1) * BL]                      # [512, 2048]
        x2t = np.ascontiguousarray(
            xc[:, 1::2].T.reshape(KT_IN, 128, BL).transpose(1, 0, 2)
        ).reshape(128, KT_IN * BL).astype(BF16)
        xloc = np.ascontiguousarray(xc).reshape(NBT, 128, HALF, 2)
        in_maps.append({
            "x2t": x2t, "xloc": xloc,
            "win": win, "wh": wh, "wout": wout,
            "bin": bin_, "bh": bh, "bout": bout,
        })
    return in_maps


def _get_nc():
    key = f"nc_{MODE}"
    if key not in _CACHE:
        _CACHE[key] = _build_nc_fp8() if MODE == "fp8" else _build_nc()
    return _CACHE[key]


def marshal(x, W_in, b_in, W_h, b_h, W_out, b_out):
    fn = _marshal_fp8 if MODE == "fp8" else _marshal
    return fn(x, W_in, b_in, W_h, b_h, W_out, b_out)


def _ensure_ntff_hook():
    """Provide antenv.axon_hooks if the image lacks it (profiling only)."""
    import sys
    import types
    try:
        from antenv.axon_hooks import get_axon_ntff_profile_hook  # noqa: F401
        return
    except ImportError:
        pass
    from trn_agent_boot.trn_boot import _ntff_profile_via_ctypes

    hook = _ntff_profile_via_ctypes("/opt/axon/libaxon_pjrt.so")
    mod = types.ModuleType("antenv.axon_hooks")
    mod.get_axon_ntff_profile_hook = lambda: hook
    mod.set_axon_ntff_profile_hook = lambda h: None
    sys.modules["antenv.axon_hooks"] = mod


def run_on_hw(in_maps, trace=False, **kw):
    from concourse import bass_utils

    if trace:
        _ensure_ntff_hook()
        bass_utils.upload_artifacts = lambda d: d  # no remote bucket here
    nc = _get_nc()
    return bass_utils.run_bass_kernel_spmd(
        nc, in_maps, core_ids=list(range(NCORES)), trace=trace, **kw
    )


def kernel(x, log_det_J, W_in, b_in, W_h, b_h, W_out, b_out):
    in_maps = marshal(x, W_in, b_in, W_h, b_h, W_out, b_out)
    res = run_on_hw(in_maps)
    y = np.concatenate(
        [res.results[c]["out"].reshape(BL, D) for c in range(NCORES)], axis=0
    )
    return y, np.asarray(log_det_J, dtype=np.float32)


# revision 20
# speedup vs baseline: 1.1980x; 1.1980x over previous
"""Trainium2 Bass kernel for an additive-coupling layer (NICE-style).

Reference math (mask_config=1, forward):
    x:[4096,2048] -> x1 = x[:, 0::2], x2 = x[:, 1::2]           (each [4096,1024])
    h = relu(x2 @ W_in + b_in)                                   [4096,4096]
    h = relu(h @ W_h[i] + b_h[i])  for i in 0..3
    out = h @ W_out + b_out                                      [4096,1024]
    y[:, 0::2] = x1 + out ; y[:, 1::2] = x2
    returns (y, log_det_J)  (log_det_J passes through unchanged)

Distribution: pure data-parallel over the batch. Each of the 8 NeuronCores
gets 512 rows and a replicated copy of the weights; no collectives.

Per-core kernel (MODE="fp8", the default): activations stay feature-major
(features on SBUF partitions, batch on the free dim) so every layer is a
chain of fp8e4m3 DoubleRow matmuls — 128x256 stationary (weight pair-tile)
x 256x512 moving (activation) — accumulating fp32 in PSUM at 2 MACs per PE
cell per cycle (157 TF/s peak). Weights are quantized with power-of-two
scales and pre-arranged on the host into per-output-tile strips so each
strip is one contiguous DMA; the ScalarE drain fuses bias + rescale + relu
+ fp8 quantization of the next layer's input. The output layer instead
makes the activations stationary so its PSUM result lands batch-major;
one DVE op per tile fuses the descale with the x1 add (b_out is folded
into the host-marshaled x1), and the y rows DMA out per batch-tile. A
burst of identity transposes at t=0 warms the PE clock gate (HAM) while
the first DMAs land.

Measured on one TRN2 chip: ~526 us NEFF exec (vs ~494 us pure-matmul
floor at fp8 peak); the bf16 variant (MODE="bf16") measures ~1059 us.
fp8 end-to-end error on y is ~4e-4 relative: the coupling output rides
on the exact x1 passthrough at ~1% of its scale, diluting GEMM error.
"""

import numpy as np
import ml_dtypes

BF16 = ml_dtypes.bfloat16

B = 4096
D = 2048
HALF = 1024
MID = 4096
N_HIDDEN = 4
NCORES = 8
BL = B // NCORES          # 512 rows per core
KT_IN = HALF // 128       # 8   k-tiles, input layer
KT_MID = MID // 128       # 32  k-tiles, hidden layers
MT_MID = MID // 128       # 32  m-tiles (output feature tiles), hidden layers
MT_OUT = HALF // 128      # 8   m-tiles, output layer
NBT = BL // 128           # 4   batch tiles per core

_CACHE = {}

# fp8 (e4m3, DoubleRow) quantization scales — powers of two. Weight tensors
# scale to absmax 128; activations to absmax <= ~100 (TRN fp8 overflows to
# Inf at 240, so keep >2x headroom). Derived from the fp32 forward pass.
SW_IN = 4096.0
SW_H = 8192.0
SW_OUT = 8192.0
SA = [16.0, 32.0, 64.0, 128.0, 256.0, 512.0]
KT2_IN = HALF // 256      # 4   k-pair-tiles, input layer
KT2_MID = MID // 256      # 16  k-pair-tiles, hidden layers
FP8 = ml_dtypes.float8_e4m3

MODE = "fp8"              # "fp8" or "bf16"


def _build_nc_fp8():
    """fp8e4m3 DoubleRow variant: 2 fp8 MACs per PE cell per cycle.

    Logical contraction index k_eff = kt*256 + j*128 + p maps to
    (k-pair-tile kt, pair j, partition p); lhsT/rhs APs are [128, 2, F]
    with the pair on dim 1 (= two adjacent 128-feature tiles), so the
    producing layer's per-m-tile ACT drain writes stay contiguous.
    """
    from contextlib import ExitStack

    import concourse.bacc as bacc
    import concourse.mybir as mybir
    import concourse.tile as tile

    f32 = mybir.dt.float32
    fp8 = mybir.dt.float8e4
    AF = mybir.ActivationFunctionType
    DR = mybir.MatmulPerfMode.DoubleRow

    nc = bacc.Bacc()

    x2t_d = nc.declare_dram_parameter("x2t", [128, KT2_IN, 2, BL], fp8, isOutput=False)
    xloc_d = nc.declare_dram_parameter("xloc", [NBT, 128, HALF, 2], f32, isOutput=False)
    win_d = nc.declare_dram_parameter("win", [MT_MID // 2, 128, 2, KT2_IN, 2, 128], fp8, isOutput=False)
    wh_d = nc.declare_dram_parameter("wh", [N_HIDDEN * MT_MID, 128, KT2_MID, 2, 128], fp8, isOutput=False)
    woutt_d = nc.declare_dram_parameter("woutt", [128, KT2_MID, 2, HALF], fp8, isOutput=False)
    bin_d = nc.declare_dram_parameter("bin", [128, MT_MID], f32, isOutput=False)
    bh_d = nc.declare_dram_parameter("bh", [128, N_HIDDEN * MT_MID], f32, isOutput=False)
    ident_d = nc.declare_dram_parameter("ident", [128, 128], f32, isOutput=False)
    out_d = nc.declare_dram_parameter("out", [NBT, 128, HALF, 2], f32, isOutput=True)

    with tile.TileContext(nc) as tc, ExitStack() as ctx:
        const = ctx.enter_context(tc.tile_pool(name="const", bufs=1))
        xp = ctx.enter_context(tc.tile_pool(name="xp", bufs=1))
        hp = ctx.enter_context(tc.tile_pool(name="hp", bufs=2))
        wp = ctx.enter_context(tc.tile_pool(name="wp", bufs=6))
        op = ctx.enter_context(tc.tile_pool(name="op", bufs=1))
        pp = ctx.enter_context(tc.tile_pool(name="pp", bufs=6, space="PSUM"))
        tp = ctx.enter_context(tc.tile_pool(name="tp", bufs=1, space="PSUM"))

        # identity comes via DMA: gpsimd (make_identity) pays a ~7us library
        # load at kernel start, which would delay the PE warmup.
        ident = const.tile([128, 128], f32)
        nc.sync.dma_start(ident[:], ident_d[:])
        x2t = const.tile([128, KT2_IN, 2, BL], fp8)
        nc.sync.dma_start(x2t[:], x2t_d[:])
        bins = const.tile([128, MT_MID], f32)
        nc.sync.dma_start(bins[:], bin_d[:])
        bhs = const.tile([128, N_HIDDEN * MT_MID], f32)
        nc.sync.dma_start(bhs[:], bh_d[:])
        # Warm the PE (HAM un-throttles after ~3.4us of sustained activity)
        # with dependency-free identity transposes while input DMAs land.
        warm = tp.tile([128, 128], f32, name="warm", tag="tp")
        for _ in range(8):
            nc.tensor.transpose(warm[:], ident[:], ident[:])

        # input layer
        sc0 = SA[1] / (SW_IN * SA[0])
        h = hp.tile([128, KT2_MID, 2, BL], fp8, name="h0", tag="h")
        for mp in range(MT_MID // 2):
            w = wp.tile([128, 2, KT2_IN, 2, 128], fp8, name=f"w0_{mp}", tag="w")
            nc.sync.dma_start(w[:], win_d[mp])
            for s_ in range(2):
                mt = 2 * mp + s_
                ps = pp.tile([128, BL], f32, name=f"ps0_{mt}", tag="ps")
                for kt in range(KT2_IN):
                    nc.tensor.matmul(
                        ps[:], w[:, s_, kt], x2t[:, kt],
                        start=(kt == 0), stop=(kt == KT2_IN - 1), perf_mode=DR,
                    )
                nc.scalar.activation(
                    h[:, mt // 2, mt % 2, :], ps[:], AF.Relu,
                    bias=bins[:, mt:mt + 1], scale=sc0,
                )

        # x rows are only needed for the final add — spread the four loads
        # across the hidden layers so they never delay a weight-strip DMA.
        xsb = xp.tile([128, NBT, HALF, 2], f32)

        woutt = op.tile([128, KT2_MID, 2, HALF], fp8)

        # hidden layers
        for layer in range(N_HIDDEN):
            nc.sync.dma_start(xsb[:, layer], xloc_d[layer])
            if layer == 1:
                nc.sync.dma_start(woutt[:], woutt_d[:])
            scl = SA[layer + 2] / (SW_H * SA[layer + 1])
            h2 = hp.tile([128, KT2_MID, 2, BL], fp8, name=f"h{layer + 1}", tag="h")
            for mt in range(MT_MID):
                lm = layer * MT_MID + mt
                w = wp.tile([128, KT2_MID, 2, 128], fp8, name=f"wh{lm}", tag="w")
                nc.sync.dma_start(w[:], wh_d[lm])
                ps = pp.tile([128, BL], f32, name=f"ps{layer + 1}_{mt}", tag="ps")
                for kt in range(KT2_MID):
                    nc.tensor.matmul(
                        ps[:], w[:, kt], h[:, kt],
                        start=(kt == 0), stop=(kt == KT2_MID - 1), perf_mode=DR,
                    )
                nc.scalar.activation(
                    h2[:, mt // 2, mt % 2, :], ps[:], AF.Relu,
                    bias=bhs[:, lm:lm + 1], scale=scl,
                )
            h = h2

        # output layer, batch-major: activations become the stationary
        # operand so psum[b, f] needs no transpose. b_out is pre-added into
        # the host-marshaled x1 columns; the PSUM descale and the x1 add are
        # fused into one DVE scalar_tensor_tensor per (bt, chunk).
        sco = 1.0 / (SW_OUT * SA[5])
        for bt in range(NBT):
            for c in range(2):
                ps = pp.tile([128, BL], f32, name=f"pso_{bt}_{c}", tag="ps")
                for kt in range(KT2_MID):
                    nc.tensor.matmul(
                        ps[:],
                        h[:, kt, :, bt * 128:(bt + 1) * 128],
                        woutt[:, kt, :, c * 512:(c + 1) * 512],
                        start=(kt == 0), stop=(kt == KT2_MID - 1), perf_mode=DR,
                    )
                dst = xsb[:, bt, c * 512:(c + 1) * 512, 0]
                nc.vector.scalar_tensor_tensor(
                    dst, ps[:], sco, dst,
                    op0=mybir.AluOpType.mult, op1=mybir.AluOpType.add,
                )
                nc.sync.dma_start(
                    out_d[bt, :, c * 512:(c + 1) * 512, :],
                    xsb[:, bt, c * 512:(c + 1) * 512, :],
                )

    nc.compile()
    return nc


def _q8(v):
    return np.clip(v, -240.0, 240.0).astype(FP8)


def _marshal_fp8(x, W_in, b_in, W_h, b_h, W_out, b_out):
    """fp8 host prep: quantized pair-block weight strips + input shards."""
    x = np.ascontiguousarray(np.asarray(x, dtype=np.float32))
    W_in = np.asarray(W_in, dtype=np.float32)
    b_in = np.asarray(b_in, dtype=np.float32)
    W_h = np.asarray(W_h, dtype=np.float32)
    b_h = np.asarray(b_h, dtype=np.float32)
    W_out = np.asarray(W_out, dtype=np.float32)
    b_out = np.asarray(b_out, dtype=np.float32)

    # strips: [mt, partition=k, kt, pair j, m]; element = W[kt*256+j*128+k, mt*128+m]
    win = _q8(np.ascontiguousarray(
        (W_in * SW_IN).reshape(KT2_IN, 2, 128, MT_MID // 2, 2, 128)
        .transpose(3, 2, 4, 0, 1, 5)
    ))
    wh = _q8(np.ascontiguousarray(
        (W_h * SW_H).reshape(N_HIDDEN, KT2_MID, 2, 128, MT_MID, 128)
        .transpose(0, 4, 3, 1, 2, 5)
    ).reshape(N_HIDDEN * MT_MID, 128, KT2_MID, 2, 128))
    woutt = _q8(np.ascontiguousarray(
        (W_out * SW_OUT).reshape(KT2_MID, 2, 128, HALF).transpose(2, 0, 1, 3)
    ))
    bin_ = np.ascontiguousarray((b_in * SA[1]).reshape(MT_MID, 128).T)
    bh = np.ascontiguousarray(
        (b_h * np.array(SA[2:6], np.float32)[:, None]).reshape(N_HIDDEN * MT_MID, 128).T
    )
    ident = np.eye(128, dtype=np.float32)

    in_maps = []
    for c in range(NCORES):
        xc = x[c * BL:(c + 1) * BL].copy()
        x2t = _q8(np.ascontiguousarray(
            (xc[:, 1::2] * SA[0]).T.reshape(KT2_IN, 2, 128, BL).transpose(2, 0, 1, 3)
        ))
        xc[:, 0::2] += b_out          # fold output bias into the x1 passthrough
        xloc = np.ascontiguousarray(xc).reshape(NBT, 128, HALF, 2)
        in_maps.append({
            "x2t": x2t, "xloc": xloc,
            "win": win, "wh": wh, "woutt": woutt,
            "bin": bin_, "bh": bh, "ident": ident,
        })
    return in_maps


def _build_nc():
    """Build the (single, SPMD-identical) Bass graph for one core."""
    from contextlib import ExitStack

    import concourse.bacc as bacc
    import concourse.mybir as mybir
    import concourse.tile as tile
    from concourse import masks

    f32 = mybir.dt.float32
    bf16 = mybir.dt.bfloat16
    AF = mybir.ActivationFunctionType

    nc = bacc.Bacc()

    x2t_d = nc.declare_dram_parameter("x2t", [128, KT_IN * BL], bf16, isOutput=False)
    xloc_d = nc.declare_dram_parameter("xloc", [NBT, 128, HALF, 2], f32, isOutput=False)
    win_d = nc.declare_dram_parameter("win", [MT_MID, 128, KT_IN * 128], bf16, isOutput=False)
    wh_d = nc.declare_dram_parameter("wh", [N_HIDDEN * MT_MID, 128, KT_MID * 128], bf16, isOutput=False)
    wout_d = nc.declare_dram_parameter("wout", [MT_OUT, 128, KT_MID * 128], bf16, isOutput=False)
    bin_d = nc.declare_dram_parameter("bin", [128, MT_MID], f32, isOutput=False)
    bh_d = nc.declare_dram_parameter("bh", [128, N_HIDDEN * MT_MID], f32, isOutput=False)
    bout_d = nc.declare_dram_parameter("bout", [128, MT_OUT], f32, isOutput=False)
    out_d = nc.declare_dram_parameter("out", [NBT, 128, HALF, 2], f32, isOutput=True)

    with tile.TileContext(nc) as tc, ExitStack() as ctx:
        const = ctx.enter_context(tc.tile_pool(name="const", bufs=1))
        xp = ctx.enter_context(tc.tile_pool(name="xp", bufs=1))
        hp = ctx.enter_context(tc.tile_pool(name="hp", bufs=2))
        wp = ctx.enter_context(tc.tile_pool(name="wp", bufs=3))
        op = ctx.enter_context(tc.tile_pool(name="op", bufs=1))
        pp = ctx.enter_context(tc.tile_pool(name="pp", bufs=4, space="PSUM"))
        tp = ctx.enter_context(tc.tile_pool(name="tp", bufs=2, space="PSUM"))

        x2t = const.tile([128, KT_IN * BL], bf16)
        nc.sync.dma_start(x2t[:], x2t_d[:])
        xsb = xp.tile([128, NBT, HALF, 2], f32)
        for bt in range(NBT):
            nc.sync.dma_start(xsb[:, bt], xloc_d[bt])
        bins = const.tile([128, MT_MID], f32)
        nc.sync.dma_start(bins[:], bin_d[:])
        bhs = const.tile([128, N_HIDDEN * MT_MID], f32)
        nc.sync.dma_start(bhs[:], bh_d[:])
        bouts = const.tile([128, MT_OUT], f32)
        nc.sync.dma_start(bouts[:], bout_d[:])
        ident = const.tile([128, 128], f32)
        masks.make_identity(nc, ident[:])

        # input layer: h = relu(x2 @ W_in + b_in), feature-major
        h = hp.tile([128, MT_MID * BL], bf16, name="h0", tag="h")
        for mt in range(MT_MID):
            w = wp.tile([128, KT_IN * 128], bf16, name=f"w0_{mt}", tag="w")
            nc.sync.dma_start(w[:], win_d[mt])
            ps = pp.tile([128, BL], f32, name=f"ps0_{mt}", tag="ps")
            for kt in range(KT_IN):
                nc.tensor.matmul(
                    ps[:],
                    w[:, kt * 128:(kt + 1) * 128],
                    x2t[:, kt * BL:(kt + 1) * BL],
                    start=(kt == 0),
                    stop=(kt == KT_IN - 1),
                )
            nc.scalar.activation(
                h[:, mt * BL:(mt + 1) * BL], ps[:], AF.Relu, bias=bins[:, mt:mt + 1]
            )

        # hidden layers
        for layer in range(N_HIDDEN):
            h2 = hp.tile([128, MT_MID * BL], bf16, name=f"h{layer + 1}", tag="h")
            for mt in range(MT_MID):
                lm = layer * MT_MID + mt
                w = wp.tile([128, KT_MID * 128], bf16, name=f"wh{lm}", tag="w")
                nc.sync.dma_start(w[:], wh_d[lm])
                ps = pp.tile([128, BL], f32, name=f"ps{layer + 1}_{mt}", tag="ps")
                for kt in range(KT_MID):
                    nc.tensor.matmul(
                        ps[:],
                        w[:, kt * 128:(kt + 1) * 128],
                        h[:, kt * BL:(kt + 1) * BL],
                        start=(kt == 0),
                        stop=(kt == KT_MID - 1),
                    )
                nc.scalar.activation(
                    h2[:, mt * BL:(mt + 1) * BL], ps[:], AF.Relu, bias=bhs[:, lm:lm + 1]
                )
            h = h2

        # output layer: out = h @ W_out + b_out (no relu), fp32, feature-major
        outT = op.tile([128, MT_OUT * BL], f32)
        for mt in range(MT_OUT):
            w = wp.tile([128, KT_MID * 128], bf16, name=f"wo_{mt}", tag="w")
            nc.sync.dma_start(w[:], wout_d[mt])
            ps = pp.tile([128, BL], f32, name=f"pso_{mt}", tag="ps")
            for kt in range(KT_MID):
                nc.tensor.matmul(
                    ps[:],
                    w[:, kt * 128:(kt + 1) * 128],
                    h[:, kt * BL:(kt + 1) * BL],
                    start=(kt == 0),
                    stop=(kt == KT_MID - 1),
                )
            nc.scalar.activation(
                outT[:, mt * BL:(mt + 1) * BL], ps[:], AF.Identity,
                bias=bouts[:, mt:mt + 1],
            )

        # transpose out back to batch-major and add into even columns of x
        for mt in range(MT_OUT):
            for bt in range(NBT):
                t = tp.tile([128, 128], f32, name=f"t{mt}_{bt}", tag="tp")
                nc.tensor.transpose(
                    t[:], outT[:, mt * BL + bt * 128: mt * BL + (bt + 1) * 128],
                    ident[:],
                )
                dst = xsb[:, bt, mt * 128:(mt + 1) * 128, 0]
                nc.vector.tensor_add(dst, dst, t[:])

        for bt in range(NBT):
            nc.sync.dma_start(out_d[bt], xsb[:, bt])

    nc.compile()
    return nc


def _marshal(x, W_in, b_in, W_h, b_h, W_out, b_out):
    """Host-side layout prep: bf16 weight strips + per-core input shards."""
    x = np.ascontiguousarray(np.asarray(x, dtype=np.float32))
    W_in = np.asarray(W_in, dtype=np.float32)
    b_in = np.asarray(b_in, dtype=np.float32)
    W_h = np.asarray(W_h, dtype=np.float32)
    b_h = np.asarray(b_h, dtype=np.float32)
    W_out = np.asarray(W_out, dtype=np.float32)
    b_out = np.asarray(b_out, dtype=np.float32)

    # weight strips: strip[mt] laid out [partition=k_in, k_tile, m_in]
    win = np.ascontiguousarray(
        W_in.reshape(KT_IN, 128, MT_MID, 128).transpose(2, 1, 0, 3)
    ).reshape(MT_MID, 128, KT_IN * 128).astype(BF16)
    wh = np.ascontiguousarray(
        W_h.reshape(N_HIDDEN, KT_MID, 128, MT_MID, 128).transpose(0, 3, 2, 1, 4)
    ).reshape(N_HIDDEN * MT_MID, 128, KT_MID * 128).astype(BF16)
    wout = np.ascontiguousarray(
        W_out.reshape(KT_MID, 128, MT_OUT, 128).transpose(2, 1, 0, 3)
    ).reshape(MT_OUT, 128, KT_MID * 128).astype(BF16)
    bin_ = np.ascontiguousarray(b_in.reshape(MT_MID, 128).T)
    bh = np.ascontiguousarray(b_h.reshape(N_HIDDEN * MT_MID, 128).T)
    bout = np.ascontiguousarray(b_out.reshape(MT_OUT, 128).T)

    in_maps = []
    for c in range(NCORES):
        xc = x[c * BL:(c + 1) * BL]                      # [512, 2048]
        x2t = np.ascontiguousarray(
            xc[:, 1::2].T.reshape(KT_IN, 128, BL).transpose(1, 0, 2)
        ).reshape(128, KT_IN * BL).astype(BF16)
        xloc = np.ascontiguousarray(xc).reshape(NBT, 128, HALF, 2)
        in_maps.append({
            "x2t": x2t, "xloc": xloc,
            "win": win, "wh": wh, "wout": wout,
            "bin": bin_, "bh": bh, "bout": bout,
        })
    return in_maps


def _get_nc():
    key = f"nc_{MODE}"
    if key not in _CACHE:
        _CACHE[key] = _build_nc_fp8() if MODE == "fp8" else _build_nc()
    return _CACHE[key]


def marshal(x, W_in, b_in, W_h, b_h, W_out, b_out):
    fn = _marshal_fp8 if MODE == "fp8" else _marshal
    return fn(x, W_in, b_in, W_h, b_h, W_out, b_out)


def _ensure_ntff_hook():
    """Provide antenv.axon_hooks if the image lacks it (profiling only)."""
    import sys
    import types
    try:
        from antenv.axon_hooks import get_axon_ntff_profile_hook  # noqa: F401
        return
    except ImportError:
        pass
    from trn_agent_boot.trn_boot import _ntff_profile_via_ctypes

    hook = _ntff_profile_via_ctypes("/opt/axon/libaxon_pjrt.so")
    mod = types.ModuleType("antenv.axon_hooks")
    mod.get_axon_ntff_profile_hook = lambda: hook
    mod.set_axon_ntff_profile_hook = lambda h: None
    sys.modules["antenv.axon_hooks"] = mod


def run_on_hw(in_maps, trace=False, **kw):
    from concourse import bass_utils

    if trace:
        _ensure_ntff_hook()
        bass_utils.upload_artifacts = lambda d: d  # no remote bucket here
    nc = _get_nc()
    return bass_utils.run_bass_kernel_spmd(
        nc, in_maps, core_ids=list(range(NCORES)), trace=trace, **kw
    )


def kernel(x, log_det_J, W_in, b_in, W_h, b_h, W_out, b_out):
    in_maps = marshal(x, W_in, b_in, W_h, b_h, W_out, b_out)
    res = run_on_hw(in_maps)
    y = np.concatenate(
        [res.results[c]["out"].reshape(BL, D) for c in range(NCORES)], axis=0
    )
    return y, np.asarray(log_det_J, dtype=np.float32)


# revision 22
# speedup vs baseline: 1.2037x; 1.0047x over previous
"""Trainium2 Bass kernel for an additive-coupling layer (NICE-style).

Reference math (mask_config=1, forward):
    x:[4096,2048] -> x1 = x[:, 0::2], x2 = x[:, 1::2]           (each [4096,1024])
    h = relu(x2 @ W_in + b_in)                                   [4096,4096]
    h = relu(h @ W_h[i] + b_h[i])  for i in 0..3
    out = h @ W_out + b_out                                      [4096,1024]
    y[:, 0::2] = x1 + out ; y[:, 1::2] = x2
    returns (y, log_det_J)  (log_det_J passes through unchanged)

Distribution: pure data-parallel over the batch. Each of the 8 NeuronCores
gets 512 rows and a replicated copy of the weights; no collectives.

Per-core kernel (MODE="fp8", the default): activations stay feature-major
(features on SBUF partitions, batch on the free dim) so every layer is a
chain of fp8e4m3 DoubleRow matmuls — 128x256 stationary (weight pair-tile)
x 256x512 moving (activation) — accumulating fp32 in PSUM at 2 MACs per PE
cell per cycle (157 TF/s peak). Weights are quantized with power-of-two
scales and pre-arranged on the host into per-output-tile strips so each
strip is one contiguous DMA; the ScalarE drain fuses bias + rescale + relu
+ fp8 quantization of the next layer's input. The output layer instead
makes the activations stationary so its PSUM result lands batch-major;
one DVE op per tile fuses the descale with the x1 add (b_out is folded
into the host-marshaled x1), and the y rows DMA out per batch-tile
as each half finishes. Startup orders the DMA ring so the first matmul
waits only on x2t's first chunk and the first W_in pair-strip; L0's own
back-to-back matmuls warm the PE clock gate (HAM).

Measured on one TRN2 chip: ~526 us NEFF exec (vs ~494 us pure-matmul
floor at fp8 peak); the bf16 variant (MODE="bf16") measures ~1059 us.
fp8 end-to-end error on y is ~4e-4 relative: the coupling output rides
on the exact x1 passthrough at ~1% of its scale, diluting GEMM error.
"""

import numpy as np
import ml_dtypes

BF16 = ml_dtypes.bfloat16

B = 4096
D = 2048
HALF = 1024
MID = 4096
N_HIDDEN = 4
NCORES = 8
BL = B // NCORES          # 512 rows per core
KT_IN = HALF // 128       # 8   k-tiles, input layer
KT_MID = MID // 128       # 32  k-tiles, hidden layers
MT_MID = MID // 128       # 32  m-tiles (output feature tiles), hidden layers
MT_OUT = HALF // 128      # 8   m-tiles, output layer
NBT = BL // 128           # 4   batch tiles per core

_CACHE = {}

# fp8 (e4m3, DoubleRow) quantization scales — powers of two. Weight tensors
# scale to absmax 128; activations to absmax <= ~100 (TRN fp8 overflows to
# Inf at 240, so keep >2x headroom). Derived from the fp32 forward pass.
SW_IN = 4096.0
SW_H = 8192.0
SW_OUT = 8192.0
SA = [16.0, 32.0, 64.0, 128.0, 256.0, 512.0]
KT2_IN = HALF // 256      # 4   k-pair-tiles, input layer
KT2_MID = MID // 256      # 16  k-pair-tiles, hidden layers
FP8 = ml_dtypes.float8_e4m3

MODE = "fp8"              # "fp8" or "bf16"


def _build_nc_fp8():
    """fp8e4m3 DoubleRow variant: 2 fp8 MACs per PE cell per cycle.

    Logical contraction index k_eff = kt*256 + j*128 + p maps to
    (k-pair-tile kt, pair j, partition p); lhsT/rhs APs are [128, 2, F]
    with the pair on dim 1 (= two adjacent 128-feature tiles), so the
    producing layer's per-m-tile ACT drain writes stay contiguous.
    """
    from contextlib import ExitStack

    import concourse.bacc as bacc
    import concourse.mybir as mybir
    import concourse.tile as tile

    f32 = mybir.dt.float32
    fp8 = mybir.dt.float8e4
    AF = mybir.ActivationFunctionType
    DR = mybir.MatmulPerfMode.DoubleRow

    nc = bacc.Bacc()

    x2t_d = nc.declare_dram_parameter("x2t", [128, KT2_IN, 2, BL], fp8, isOutput=False)
    xloc_d = nc.declare_dram_parameter("xloc", [NBT, 128, HALF, 2], f32, isOutput=False)
    win_d = nc.declare_dram_parameter("win", [MT_MID // 2, 128, 2, KT2_IN, 2, 128], fp8, isOutput=False)
    wh_d = nc.declare_dram_parameter("wh", [N_HIDDEN * MT_MID, 128, KT2_MID, 2, 128], fp8, isOutput=False)
    woutt_d = nc.declare_dram_parameter("woutt", [128, KT2_MID, 2, HALF], fp8, isOutput=False)
    bin_d = nc.declare_dram_parameter("bin", [128, MT_MID], f32, isOutput=False)
    bh_d = nc.declare_dram_parameter("bh", [128, N_HIDDEN * MT_MID], f32, isOutput=False)
    out_d = nc.declare_dram_parameter("out", [NBT, 128, HALF, 2], f32, isOutput=True)

    with tile.TileContext(nc) as tc, ExitStack() as ctx:
        const = ctx.enter_context(tc.tile_pool(name="const", bufs=1))
        xp = ctx.enter_context(tc.tile_pool(name="xp", bufs=1))
        hp = ctx.enter_context(tc.tile_pool(name="hp", bufs=2))
        wp = ctx.enter_context(tc.tile_pool(name="wp", bufs=6))
        op = ctx.enter_context(tc.tile_pool(name="op", bufs=1))
        pp = ctx.enter_context(tc.tile_pool(name="pp", bufs=6, space="PSUM"))

        # Startup-critical ring order: the first matmul needs only x2t's
        # kt=0 chunk and the first W_in pair-strip, so those go first; the
        # rest of x2t and the biases follow. No PE warmup burst — L0's own
        # back-to-back matmuls un-throttle the HAM clock gate while the
        # strip DMAs stay ahead of the PE.
        x2t = const.tile([128, KT2_IN, 2, BL], fp8)
        nc.sync.dma_start(x2t[:, 0], x2t_d[:, 0])
        bins = const.tile([128, MT_MID], f32)
        bhs = const.tile([128, N_HIDDEN * MT_MID], f32)
        bhs_src = bh_d

        # input layer
        sc0 = SA[1] / (SW_IN * SA[0])
        h = hp.tile([128, KT2_MID, 2, BL], fp8, name="h0", tag="h")
        for mp in range(MT_MID // 2):
            w = wp.tile([128, 2, KT2_IN, 2, 128], fp8, name=f"w0_{mp}", tag="w")
            nc.sync.dma_start(w[:], win_d[mp])
            if mp == 0:
                for kt in range(1, KT2_IN):
                    nc.sync.dma_start(x2t[:, kt], x2t_d[:, kt])
                nc.sync.dma_start(bins[:], bin_d[:])
                nc.sync.dma_start(bhs[:], bhs_src[:])
            for s_ in range(2):
                mt = 2 * mp + s_
                ps = pp.tile([128, BL], f32, name=f"ps0_{mt}", tag="ps")
                for kt in range(KT2_IN):
                    nc.tensor.matmul(
                        ps[:], w[:, s_, kt], x2t[:, kt],
                        start=(kt == 0), stop=(kt == KT2_IN - 1), perf_mode=DR,
                    )
                nc.scalar.activation(
                    h[:, mt // 2, mt % 2, :], ps[:], AF.Relu,
                    bias=bins[:, mt:mt + 1], scale=sc0,
                )

        # x rows are only needed for the final add — spread the four loads
        # across the hidden layers so they never delay a weight-strip DMA.
        xsb = xp.tile([128, NBT, HALF, 2], f32)

        woutt = op.tile([128, KT2_MID, 2, HALF], fp8)

        # hidden layers
        for layer in range(N_HIDDEN):
            nc.sync.dma_start(xsb[:, layer], xloc_d[layer])
            if layer == 1:
                nc.sync.dma_start(woutt[:], woutt_d[:])
            scl = SA[layer + 2] / (SW_H * SA[layer + 1])
            h2 = hp.tile([128, KT2_MID, 2, BL], fp8, name=f"h{layer + 1}", tag="h")
            for mt in range(MT_MID):
                lm = layer * MT_MID + mt
                w = wp.tile([128, KT2_MID, 2, 128], fp8, name=f"wh{lm}", tag="w")
                nc.sync.dma_start(w[:], wh_d[lm])
                ps = pp.tile([128, BL], f32, name=f"ps{layer + 1}_{mt}", tag="ps")
                for kt in range(KT2_MID):
                    nc.tensor.matmul(
                        ps[:], w[:, kt], h[:, kt],
                        start=(kt == 0), stop=(kt == KT2_MID - 1), perf_mode=DR,
                    )
                nc.scalar.activation(
                    h2[:, mt // 2, mt % 2, :], ps[:], AF.Relu,
                    bias=bhs[:, lm:lm + 1], scale=scl,
                )
            h = h2

        # output layer, batch-major: activations become the stationary
        # operand so psum[b, f] needs no transpose. b_out is pre-added into
        # the host-marshaled x1 columns; the PSUM descale and the x1 add are
        # fused into one DVE scalar_tensor_tensor per (bt, chunk).
        sco = 1.0 / (SW_OUT * SA[5])
        for bt in range(NBT):
            for c in range(2):
                ps = pp.tile([128, BL], f32, name=f"pso_{bt}_{c}", tag="ps")
                for kt in range(KT2_MID):
                    nc.tensor.matmul(
                        ps[:],
                        h[:, kt, :, bt * 128:(bt + 1) * 128],
                        woutt[:, kt, :, c * 512:(c + 1) * 512],
                        start=(kt == 0), stop=(kt == KT2_MID - 1), perf_mode=DR,
                    )
                dst = xsb[:, bt, c * 512:(c + 1) * 512, 0]
                nc.vector.scalar_tensor_tensor(
                    dst, ps[:], sco, dst,
                    op0=mybir.AluOpType.mult, op1=mybir.AluOpType.add,
                )
                nc.sync.dma_start(
                    out_d[bt, :, c * 512:(c + 1) * 512, :],
                    xsb[:, bt, c * 512:(c + 1) * 512, :],
                )

    nc.compile()
    return nc


def _q8(v):
    return np.clip(v, -240.0, 240.0).astype(FP8)


def _marshal_fp8(x, W_in, b_in, W_h, b_h, W_out, b_out):
    """fp8 host prep: quantized pair-block weight strips + input shards."""
    x = np.ascontiguousarray(np.asarray(x, dtype=np.float32))
    W_in = np.asarray(W_in, dtype=np.float32)
    b_in = np.asarray(b_in, dtype=np.float32)
    W_h = np.asarray(W_h, dtype=np.float32)
    b_h = np.asarray(b_h, dtype=np.float32)
    W_out = np.asarray(W_out, dtype=np.float32)
    b_out = np.asarray(b_out, dtype=np.float32)

    # strips: [mt, partition=k, kt, pair j, m]; element = W[kt*256+j*128+k, mt*128+m]
    win = _q8(np.ascontiguousarray(
        (W_in * SW_IN).reshape(KT2_IN, 2, 128, MT_MID // 2, 2, 128)
        .transpose(3, 2, 4, 0, 1, 5)
    ))
    wh = _q8(np.ascontiguousarray(
        (W_h * SW_H).reshape(N_HIDDEN, KT2_MID, 2, 128, MT_MID, 128)
        .transpose(0, 4, 3, 1, 2, 5)
    ).reshape(N_HIDDEN * MT_MID, 128, KT2_MID, 2, 128))
    woutt = _q8(np.ascontiguousarray(
        (W_out * SW_OUT).reshape(KT2_MID, 2, 128, HALF).transpose(2, 0, 1, 3)
    ))
    bin_ = np.ascontiguousarray((b_in * SA[1]).reshape(MT_MID, 128).T)
    bh = np.ascontiguousarray(
        (b_h * np.array(SA[2:6], np.float32)[:, None]).reshape(N_HIDDEN * MT_MID, 128).T
    )
    in_maps = []
    for c in range(NCORES):
        xc = x[c * BL:(c + 1) * BL].copy()
        x2t = _q8(np.ascontiguousarray(
            (xc[:, 1::2] * SA[0]).T.reshape(KT2_IN, 2, 128, BL).transpose(2, 0, 1, 3)
        ))
        xc[:, 0::2] += b_out          # fold output bias into the x1 passthrough
        xloc = np.ascontiguousarray(xc).reshape(NBT, 128, HALF, 2)
        in_maps.append({
            "x2t": x2t, "xloc": xloc,
            "win": win, "wh": wh, "woutt": woutt,
            "bin": bin_, "bh": bh,
        })
    return in_maps


def _build_nc():
    """Build the (single, SPMD-identical) Bass graph for one core."""
    from contextlib import ExitStack

    import concourse.bacc as bacc
    import concourse.mybir as mybir
    import concourse.tile as tile
    from concourse import masks

    f32 = mybir.dt.float32
    bf16 = mybir.dt.bfloat16
    AF = mybir.ActivationFunctionType

    nc = bacc.Bacc()

    x2t_d = nc.declare_dram_parameter("x2t", [128, KT_IN * BL], bf16, isOutput=False)
    xloc_d = nc.declare_dram_parameter("xloc", [NBT, 128, HALF, 2], f32, isOutput=False)
    win_d = nc.declare_dram_parameter("win", [MT_MID, 128, KT_IN * 128], bf16, isOutput=False)
    wh_d = nc.declare_dram_parameter("wh", [N_HIDDEN * MT_MID, 128, KT_MID * 128], bf16, isOutput=False)
    wout_d = nc.declare_dram_parameter("wout", [MT_OUT, 128, KT_MID * 128], bf16, isOutput=False)
    bin_d = nc.declare_dram_parameter("bin", [128, MT_MID], f32, isOutput=False)
    bh_d = nc.declare_dram_parameter("bh", [128, N_HIDDEN * MT_MID], f32, isOutput=False)
    bout_d = nc.declare_dram_parameter("bout", [128, MT_OUT], f32, isOutput=False)
    out_d = nc.declare_dram_parameter("out", [NBT, 128, HALF, 2], f32, isOutput=True)

    with tile.TileContext(nc) as tc, ExitStack() as ctx:
        const = ctx.enter_context(tc.tile_pool(name="const", bufs=1))
        xp = ctx.enter_context(tc.tile_pool(name="xp", bufs=1))
        hp = ctx.enter_context(tc.tile_pool(name="hp", bufs=2))
        wp = ctx.enter_context(tc.tile_pool(name="wp", bufs=3))
        op = ctx.enter_context(tc.tile_pool(name="op", bufs=1))
        pp = ctx.enter_context(tc.tile_pool(name="pp", bufs=4, space="PSUM"))
        tp = ctx.enter_context(tc.tile_pool(name="tp", bufs=2, space="PSUM"))

        x2t = const.tile([128, KT_IN * BL], bf16)
        nc.sync.dma_start(x2t[:], x2t_d[:])
        xsb = xp.tile([128, NBT, HALF, 2], f32)
        for bt in range(NBT):
            nc.sync.dma_start(xsb[:, bt], xloc_d[bt])
        bins = const.tile([128, MT_MID], f32)
        nc.sync.dma_start(bins[:], bin_d[:])
        bhs = const.tile([128, N_HIDDEN * MT_MID], f32)
        nc.sync.dma_start(bhs[:], bh_d[:])
        bouts = const.tile([128, MT_OUT], f32)
        nc.sync.dma_start(bouts[:], bout_d[:])
        ident = const.tile([128, 128], f32)
        masks.make_identity(nc, ident[:])

        # input layer: h = relu(x2 @ W_in + b_in), feature-major
        h = hp.tile([128, MT_MID * BL], bf16, name="h0", tag="h")
        for mt in range(MT_MID):
            w = wp.tile([128, KT_IN * 128], bf16, name=f"w0_{mt}", tag="w")
            nc.sync.dma_start(w[:], win_d[mt])
            ps = pp.tile([128, BL], f32, name=f"ps0_{mt}", tag="ps")
            for kt in range(KT_IN):
                nc.tensor.matmul(
                    ps[:],
                    w[:, kt * 128:(kt + 1) * 128],
                    x2t[:, kt * BL:(kt + 1) * BL],
                    start=(kt == 0),
                    stop=(kt == KT_IN - 1),
                )
            nc.scalar.activation(
                h[:, mt * BL:(mt + 1) * BL], ps[:], AF.Relu, bias=bins[:, mt:mt + 1]
            )

        # hidden layers
        for layer in range(N_HIDDEN):
            h2 = hp.tile([128, MT_MID * BL], bf16, name=f"h{layer + 1}", tag="h")
            for mt in range(MT_MID):
                lm = layer * MT_MID + mt
                w = wp.tile([128, KT_MID * 128], bf16, name=f"wh{lm}", tag="w")
                nc.sync.dma_start(w[:], wh_d[lm])
                ps = pp.tile([128, BL], f32, name=f"ps{layer + 1}_{mt}", tag="ps")
                for kt in range(KT_MID):
                    nc.tensor.matmul(
                        ps[:],
                        w[:, kt * 128:(kt + 1) * 128],
                        h[:, kt * BL:(kt + 1) * BL],
                        start=(kt == 0),
                        stop=(kt == KT_MID - 1),
                    )
                nc.scalar.activation(
                    h2[:, mt * BL:(mt + 1) * BL], ps[:], AF.Relu, bias=bhs[:, lm:lm + 1]
                )
            h = h2

        # output layer: out = h @ W_out + b_out (no relu), fp32, feature-major
        outT = op.tile([128, MT_OUT * BL], f32)
        for mt in range(MT_OUT):
            w = wp.tile([128, KT_MID * 128], bf16, name=f"wo_{mt}", tag="w")
            nc.sync.dma_start(w[:], wout_d[mt])
            ps = pp.tile([128, BL], f32, name=f"pso_{mt}", tag="ps")
            for kt in range(KT_MID):
                nc.tensor.matmul(
                    ps[:],
                    w[:, kt * 128:(kt + 1) * 128],
                    h[:, kt * BL:(kt + 1) * BL],
                    start=(kt == 0),
                    stop=(kt == KT_MID - 1),
                )
            nc.scalar.activation(
                outT[:, mt * BL:(mt + 1) * BL], ps[:], AF.Identity,
                bias=bouts[:, mt:mt + 1],
            )

        # transpose out back to batch-major and add into even columns of x
        for mt in range(MT_OUT):
            for bt in range(NBT):
                t = tp.tile([128, 128], f32, name=f"t{mt}_{bt}", tag="tp")
                nc.tensor.transpose(
                    t[:], outT[:, mt * BL + bt * 128: mt * BL + (bt + 1) * 128],
                    ident[:],
                )
                dst = xsb[:, bt, mt * 128:(mt + 1) * 128, 0]
                nc.vector.tensor_add(dst, dst, t[:])

        for bt in range(NBT):
            nc.sync.dma_start(out_d[bt], xsb[:, bt])

    nc.compile()
    return nc


def _marshal(x, W_in, b_in, W_h, b_h, W_out, b_out):
    """Host-side layout prep: bf16 weight strips + per-core input shards."""
    x = np.ascontiguousarray(np.asarray(x, dtype=np.float32))
    W_in = np.asarray(W_in, dtype=np.float32)
    b_in = np.asarray(b_in, dtype=np.float32)
    W_h = np.asarray(W_h, dtype=np.float32)
    b_h = np.asarray(b_h, dtype=np.float32)
    W_out = np.asarray(W_out, dtype=np.float32)
    b_out = np.asarray(b_out, dtype=np.float32)

    # weight strips: strip[mt] laid out [partition=k_in, k_tile, m_in]
    win = np.ascontiguousarray(
        W_in.reshape(KT_IN, 128, MT_MID, 128).transpose(2, 1, 0, 3)
    ).reshape(MT_MID, 128, KT_IN * 128).astype(BF16)
    wh = np.ascontiguousarray(
        W_h.reshape(N_HIDDEN, KT_MID, 128, MT_MID, 128).transpose(0, 3, 2, 1, 4)
    ).reshape(N_HIDDEN * MT_MID, 128, KT_MID * 128).astype(BF16)
    wout = np.ascontiguousarray(
        W_out.reshape(KT_MID, 128, MT_OUT, 128).transpose(2, 1, 0, 3)
    ).reshape(MT_OUT, 128, KT_MID * 128).astype(BF16)
    bin_ = np.ascontiguousarray(b_in.reshape(MT_MID, 128).T)
    bh = np.ascontiguousarray(b_h.reshape(N_HIDDEN * MT_MID, 128).T)
    bout = np.ascontiguousarray(b_out.reshape(MT_OUT, 128).T)

    in_maps = []
    for c in range(NCORES):
        xc = x[c * BL:(c + 1) * BL]                      # [512, 2048]
        x2t = np.ascontiguousarray(
            xc[:, 1::2].T.reshape(KT_IN, 128, BL).transpose(1, 0, 2)
        ).reshape(128, KT_IN * BL).astype(BF16)
        xloc = np.ascontiguousarray(xc).reshape(NBT, 128, HALF, 2)
        in_maps.append({
            "x2t": x2t, "xloc": xloc,
            "win": win, "wh": wh, "wout": wout,
            "bin": bin_, "bh": bh, "bout": bout,
        })
    return in_maps


def _get_nc():
    key = f"nc_{MODE}"
    if key not in _CACHE:
        _CACHE[key] = _build_nc_fp8() if MODE == "fp8" else _build_nc()
    return _CACHE[key]


def marshal(x, W_in, b_in, W_h, b_h, W_out, b_out):
    fn = _marshal_fp8 if MODE == "fp8" else _marshal
    return fn(x, W_in, b_in, W_h, b_h, W_out, b_out)


def _ensure_ntff_hook():
    """Provide antenv.axon_hooks if the image lacks it (profiling only)."""
    import sys
    import types
    try:
        from antenv.axon_hooks import get_axon_ntff_profile_hook  # noqa: F401
        return
    except ImportError:
        pass
    from trn_agent_boot.trn_boot import _ntff_profile_via_ctypes

    hook = _ntff_profile_via_ctypes("/opt/axon/libaxon_pjrt.so")
    mod = types.ModuleType("antenv.axon_hooks")
    mod.get_axon_ntff_profile_hook = lambda: hook
    mod.set_axon_ntff_profile_hook = lambda h: None
    sys.modules["antenv.axon_hooks"] = mod


def run_on_hw(in_maps, trace=False, **kw):
    from concourse import bass_utils

    if trace:
        _ensure_ntff_hook()
        bass_utils.upload_artifacts = lambda d: d  # no remote bucket here
    nc = _get_nc()
    return bass_utils.run_bass_kernel_spmd(
        nc, in_maps, core_ids=list(range(NCORES)), trace=trace, **kw
    )


def kernel(x, log_det_J, W_in, b_in, W_h, b_h, W_out, b_out):
    in_maps = marshal(x, W_in, b_in, W_h, b_h, W_out, b_out)
    res = run_on_hw(in_maps)
    y = np.concatenate(
        [res.results[c]["out"].reshape(BL, D) for c in range(NCORES)], axis=0
    )
    return y, np.asarray(log_det_J, dtype=np.float32)
